# revision 28
# baseline (speedup 1.0000x reference)
"""Trainium2 Bass kernel for nn_ClockAwareGNN (segment_reduce).

Model (reference, fp32):
    gp   = segment_mean(x, batch) @ W_base + b_base            # [B, 1]
    h    = relu(clock @ W1 + b1) @ W2 + b2                     # [N, 16]
    cp   = segment_mean(h, batch)                              # [B, 16]
    out  = relu([gp | cp] @ W3 + b3) @ W4 + b4                 # [B, 1]

Everything after the segment reductions is affine in per-graph quantities, so
the heavy per-node work collapses to fused segment sums:
    Sx[g]  = sum of x rows in graph g          (128 cols)
    Sr[g]  = sum of r rows in graph g          (R cols)
where r is either the raw clock (R=1; exact when b1 == 0 and clock >= 0 since
relu(c*W1) == c*relu(W1) elementwise for c >= 0) or the host-computed
relu(clock @ W1 + b1) (R=16 fallback).  Graph node counts (and their
reciprocals) come from the sorted batch vector on the host, so no count
column is shipped.

Device strategy (per core, 8-way data-parallel by graph):
  - the whole payload is ONE fp8e4m3 tensor [x | r_hi] -> 1 B/elem of
    x traffic (the 2e-2 rel-err gate leaves ~8x margin at fp8 precision).
  - nodes arrive as 128-row tiles; batch ids are sorted so each tile touches
    <= 2 graphs inside one 32-graph "window" (4 windows per core).
  - tiles are packed ROUND-ROBIN across the 4 windows, so consecutive
    matmuls target different 32-partition PE col-groups and run concurrently
    on different sub-arrays (col tiling), ~4x PE throughput.
  - the payload DRAM layout is partition-major ([128, n_tiles*C]) so a DMA
    block of any size is one plain 2D transfer with long contiguous
    per-partition lines (33 KB at 4 supers); ramped block sizes alternate
    between the two HWDGE rings (sync/SP + scalar/ACT).
  - DVE builds one-hot assign tiles [128 nodes, 32 graphs] per super-tile in
    one is_equal op (broadcast AP vs an iota pattern).
  - PE accumulates assign.T @ payload into PSUM [128 graphs, C] fp32.
  - short fused vector-engine epilogue reads PSUM directly and computes the
    folded per-graph MLP.
"""

import math
import sys
import types

import numpy as np
import ml_dtypes

import concourse.bass as bass
import concourse.bacc as bacc
import concourse.tile as tile
from concourse import mybir
from concourse.bass_utils import run_bass_kernel_spmd


def _ensure_axon_hooks():
    """bass_utils' trace path does `from antenv.axon_hooks import ...`;
    some agent images lack that submodule. Install it (with the real NTFF
    hook when available) so trace=True degrades gracefully instead of
    raising ModuleNotFoundError."""
    try:
        import antenv  # noqa: F401
        import antenv.axon_hooks  # noqa: F401
        return
    except ImportError:
        pass
    try:
        import antenv
    except ImportError:
        return
    mod = types.ModuleType("antenv.axon_hooks")
    state = {"hook": None}
    mod.set_axon_ntff_profile_hook = lambda h: state.__setitem__("hook", h)
    mod.get_axon_ntff_profile_hook = lambda: state["hook"]
    sys.modules["antenv.axon_hooks"] = mod
    antenv.axon_hooks = mod
    try:
        from trn_agent_boot.trn_boot import _ntff_profile_via_ctypes
        mod.set_axon_ntff_profile_hook(
            _ntff_profile_via_ctypes("/opt/axon/libaxon_pjrt.so"))
    except Exception:
        pass
    # the trace path also uploads the NEFF dir to a bucket; in zero-egress
    # containers that raises — fall back to the local path.
    try:
        import concourse.bass_utils as _bu
        _orig_upload = _bu.upload_artifacts

        def _safe_upload(tmpdir):
            try:
                return _orig_upload(tmpdir)
            except Exception:
                return str(tmpdir)

        _bu.upload_artifacts = _safe_upload
    except Exception:
        pass


_ensure_axon_hooks()

BF16 = ml_dtypes.bfloat16
F8 = ml_dtypes.float8_e4m3

N_CORES = 8
N_GRAPHS = 1024
D = 128                 # feature dim of x
GPC = N_GRAPHS // N_CORES   # graphs per core = 128
W = 32                  # one-hot window width (PSUM partition alignment unit)
WPC = GPC // W          # windows per core = 4
ST = 64                 # node-tiles per super-tile (assign-op granularity)
MAXBT = 4 * ST          # steady-state DMA block size in node-tiles


def _ring_blocks(n_tiles):
    """DMA blocks in node-tile units: ramp up (so the first matmuls aren't
    gated on a multi-MB transfer), big middle blocks (fewer HBM
    write-receipt jitter events at the consumption frontier), ramp down (so
    the last matmuls start right after the last bytes land). Rings alternate
    in consumption order; the scalar ring gets block 0 (the sync ring
    carries the batch-rel ids at kernel start), so head-of-line blocking is
    bounded by one block."""
    up = [32, 32, 64, 64, 128, 128]
    down = [128, 64, 32, 32]
    sizes = []
    left = n_tiles
    for v in up:
        if left - v >= sum(down):
            sizes.append(v)
            left -= v
    ndown = [v for v in down if v <= left]
    left -= sum(ndown)
    mid = []
    while left:
        bt = min(MAXBT, left)
        mid.append(bt)
        left -= bt
    sizes += mid + ndown
    out = []
    j0 = 0
    for i, bt in enumerate(sizes):
        out.append(((i + 1) % 2, j0, bt))
        j0 += bt
    return out


def _build_program(S, C, R):
    """Build the SPMD Bass/Tile program. Shapes are static; per-core data
    differences live entirely in the input tensors.

    S: number of super-tiles (each ST node-tiles of 128 nodes)
    C: fp8 payload column count = 128 + R
    """
    fp32 = mybir.dt.float32
    bf16 = mybir.dt.bfloat16
    f8 = mybir.dt.float8e4
    n_tiles = S * ST
    blocks = _ring_blocks(n_tiles)
    # combined epilogue-constants layout: [rec|wb|v1|m2|v0|w4|bb|b4]
    NCON = 1 + D + 32 + R * 32 + 32 + 32 + 1 + 1

    nc = bacc.Bacc("TRN2", target_bir_lowering=False, debug=False,
                   num_devices=N_CORES)

    BRH = min(4 * ST, S * ST)   # brall head: enough for the first 4 supers

    xcc = nc.dram_tensor("xcc", [128, n_tiles * C], f8, kind="ExternalInput").ap()
    brs = nc.dram_tensor("brs", [128, S * ST], bf16, kind="ExternalInput").ap()
    con_b = nc.dram_tensor("con_b", [128, NCON], fp32, kind="ExternalInput").ap()
    out_d = nc.dram_tensor("out", [128, 1], fp32, kind="ExternalOutput").ap()

    with tile.TileContext(nc) as tc:
        with (
            tc.tile_pool(name="consts", bufs=1) as cpool,
            tc.tile_pool(name="xin", bufs=4) as xpool,
            tc.tile_pool(name="assign", bufs=6) as apool,
            tc.tile_pool(name="epi", bufs=1) as epool,
            tc.tile_pool(name="ps", bufs=1, space="PSUM") as ppool,
        ):
            # ---- constants: iota is generated on the (otherwise idle)
            # GpSimd engine; the batch-rel ids stream on the sync ring with a
            # small head first so the first is_equal isn't gated on the full
            # transfer, while the scalar ring streams payload block 0.
            iota_t = cpool.tile([128, W], bf16, tag="iota")
            nc.gpsimd.iota(
                iota_t[:], pattern=[[1, W]], base=0, channel_multiplier=0,
                allow_small_or_imprecise_dtypes=True,
            )
            brall = cpool.tile([128, S * ST], bf16, tag="brall")
            nc.sync.dma_start(brall[:, 0:BRH], brs[:, 0:BRH])
            nc.sync.dma_start(brall[:, BRH:], brs[:, BRH:])
            # epilogue-only constants: issued on the scalar ring AFTER its
            # first payload block (they're needed only at the very end)
            con_t = cpool.tile([128, NCON], fp32, tag="con")
            rec_t = con_t[:, 0:1]
            wb_t = con_t[:, 1 : 1 + D]
            v1_t = con_t[:, 1 + D : 33 + D]
            m2_t = con_t[:, 33 + D : 33 + D + R * 32]
            v0_t = con_t[:, 33 + D + R * 32 : 65 + D + R * 32]
            w4_t = con_t[:, 65 + D + R * 32 : 97 + D + R * 32]
            bbt = con_t[:, 97 + D + R * 32 : 98 + D + R * 32]
            b4t = con_t[:, 98 + D + R * 32 : 99 + D + R * 32]

            psum = ppool.tile([128, C], fp32, tag="acc")

            # init matmul: zero weights x zero rhs, start=True claims the
            # whole bank's has_written bits so all later matmuls (start=False)
            # overwrite-on-first-touch / accumulate-after, independent of
            # window interleaving.
            zw = cpool.tile([128, 128], bf16, tag="zw")
            nc.vector.memset(zw[:], 0.0)
            zr = cpool.tile([128, C], bf16, tag="zr")
            nc.vector.memset(zr[:], 0.0)
            nc.tensor.matmul(psum[:, :], zw[:], zr[:], start=True, stop=False)

            # ---- main loop: 2D payload DMA blocks (tile-granular, ramped),
            # one-hot assign per super-tile, matmuls per node-tile.
            rings = (nc.sync, nc.scalar)
            asgs = [None] * S
            iota_bc = (iota_t[:].rearrange("p (o j) -> p o j", o=1)
                       .to_broadcast((128, ST, W)))
            con_sent = False
            for ring_idx, j0, bt in blocks:
                xt = xpool.tile([128, MAXBT * C], f8, tag="xt")
                L = bt * C
                rings[ring_idx].dma_start(
                    xt[:, 0:L], xcc[:, j0 * C : j0 * C + L])
                if not con_sent and ring_idx == 1:
                    nc.scalar.dma_start(con_t[:], con_b)
                    con_sent = True
                for i in range(j0, j0 + bt):
                    s = i // ST
                    if asgs[s] is None:
                        # asg[p, t, j] = (iota[j] == br[p, s*ST + t])
                        asg = apool.tile([128, ST * W], bf16, tag="asg")
                        nc.vector.tensor_tensor(
                            asg[:].rearrange("p (t j) -> p t j", j=W),
                            iota_bc,
                            brall[:, s * ST : (s + 1) * ST]
                                .rearrange("p (t o) -> p t o", o=1)
                                .to_broadcast((128, ST, W)),
                            op=mybir.AluOpType.is_equal,
                        )
                        asgs[s] = asg
                    w = i % WPC        # round-robin window -> PE col-group
                    last = i == n_tiles - 1
                    off = (i - j0) * C
                    nc.tensor.matmul(
                        psum[w * W : (w + 1) * W, 0:C],
                        asgs[s][:, (i % ST) * W : (i % ST + 1) * W],
                        xt[:, off : off + C],
                        start=False,
                        stop=last,
                        tile_position=(0, w * W),
                    )

            # ---- epilogue (per-graph folded MLP), vector-only, PSUM-direct
            # gp = (Sx @ W_base) * rec + b_base
            t1 = epool.tile([128, D], fp32, tag="t1")
            nc.vector.tensor_mul(t1[:], psum[:, 0:D], wb_t)
            gp = epool.tile([128, 1], fp32, tag="gp")
            nc.vector.tensor_reduce(gp[:], t1[:], axis=mybir.AxisListType.X,
                                    op=mybir.AluOpType.add)
            nc.vector.tensor_scalar(gp[:], gp[:], rec_t, bbt,
                                    op0=mybir.AluOpType.mult,
                                    op1=mybir.AluOpType.add)
            # mr = Sr * rec
            mr = epool.tile([128, R], fp32, tag="mr")
            nc.vector.tensor_scalar_mul(mr[:], psum[:, D : D + R], rec_t)

            # pre = gp*v1 + sum_j mr[:,j]*M2[j] + v0
            pre = epool.tile([128, 32], fp32, tag="pre")
            nc.vector.scalar_tensor_tensor(pre[:], v1_t, gp[:], v0_t,
                                           op0=mybir.AluOpType.mult,
                                           op1=mybir.AluOpType.add)
            for j in range(R):
                nc.vector.scalar_tensor_tensor(
                    pre[:], m2_t[:, j * 32 : (j + 1) * 32], mr[:, j : j + 1],
                    pre[:], op0=mybir.AluOpType.mult,
                    op1=mybir.AluOpType.add)
            # out = rowsum(relu(pre) * W4) + b4
            act = epool.tile([128, 32], fp32, tag="act")
            nc.vector.tensor_scalar_max(act[:], pre[:], 0.0)
            nc.vector.tensor_mul(act[:], act[:], w4_t)
            oo = epool.tile([128, 1], fp32, tag="oo")
            nc.vector.tensor_reduce(oo[:], act[:], axis=mybir.AxisListType.X,
                                    op=mybir.AluOpType.add)
            nc.vector.tensor_add(oo[:], oo[:], b4t)

            nc.scalar.dma_start(out_d, oo[:])

    nc.compile()
    return nc


def kernel(x, clock_period, batch, W_base, b_base, W1, b1, W2, b2, W3, b3, W4, b4,
           _profile=None):
    x = np.asarray(x, np.float32)
    clock = np.asarray(clock_period, np.float32).reshape(-1)
    batch = np.asarray(batch, np.int32)
    W_base = np.asarray(W_base, np.float32)
    W1 = np.asarray(W1, np.float32); b1 = np.asarray(b1, np.float32)
    W2 = np.asarray(W2, np.float32); b2 = np.asarray(b2, np.float32)
    W3 = np.asarray(W3, np.float32); b3 = np.asarray(b3, np.float32)
    W4 = np.asarray(W4, np.float32); b4 = np.asarray(b4, np.float32)
    hid = W1.shape[1]

    # r-path: exact algebraic fold when relu(c*W1 + b1) == c * relu(W1)
    fold = bool(np.all(b1 == 0.0)) and bool(clock.min() >= 0.0)
    if fold:
        R = 1
        r32 = clock[:, None]                                   # [N, 1]
        q = np.maximum(W1, 0.0) @ W2                           # [1, hid]
        M2 = q @ W3[1:, :]                                     # [1, 32]
        v0 = b2 @ W3[1:, :] + b3                               # [32]
    else:
        R = hid
        r32 = np.maximum(clock[:, None] @ W1 + b1, 0.0)        # [N, hid]
        M2 = W2 @ W3[1:, :]                                    # [hid, 32]
        v0 = b2 @ W3[1:, :] + b3

    C = D + R               # [x | r_hi], all fp8e4m3

    # ---- shard by graph; window padding so tile->window map is static ----
    cutw = np.searchsorted(batch, np.arange(0, N_GRAPHS + 1, W))
    cutg = np.searchsorted(batch, np.arange(0, N_GRAPHS + 1))
    win_nodes = np.diff(cutw)
    T_w = int(math.ceil(win_nodes.max() / 128.0))
    while (WPC * T_w) % ST:
        T_w += 1
    n_tiles = WPC * T_w
    S = n_tiles // ST
    Npad = n_tiles * 128

    xq = x.astype(F8)
    rhi = r32.astype(F8)

    in_maps = []
    NCON = 1 + D + 32 + R * 32 + 32 + 32 + 1 + 1
    conv = np.empty(NCON - 1 - D, np.float32)  # static part after [rec|wb]
    conv[0:32] = W3[0, :]
    conv[32 : 32 + R * 32] = M2.reshape(-1)
    conv[32 + R * 32 : 64 + R * 32] = v0
    conv[64 + R * 32 : 96 + R * 32] = W4[:, 0]
    conv[96 + R * 32] = float(b_base.reshape(-1)[0])
    conv[97 + R * 32] = float(b4.reshape(-1)[0])

    for k in range(N_CORES):
        xcc = np.zeros((Npad, C), F8)
        br = np.full(Npad, -1.0, BF16)
        for wi in range(WPC):
            gw = k * WPC + wi          # global window index
            s0, e0 = int(cutw[gw]), int(cutw[gw + 1])
            n = e0 - s0
            # window wi's rows live in tiles j = wi, wi+WPC, wi+2*WPC, ...
            # (round-robin across windows); build a row-index map for them.
            rows = (np.arange(n) // 128) * (WPC * 128) + wi * 128 + (np.arange(n) % 128)
            xcc[rows, 0:D] = xq[s0:e0]
            xcc[rows, D : D + R] = rhi[s0:e0]
            br[rows] = (batch[s0:e0] - gw * W).astype(BF16)
        brs = np.ascontiguousarray(br.reshape(S * ST, 128).T)
        # partition-major: each SBUF partition's whole run is contiguous
        xcc_p = np.ascontiguousarray(
            xcc.reshape(n_tiles, 128, C).transpose(1, 0, 2)
        ).reshape(128, n_tiles * C)
        cnt_k = (cutg[k * GPC + 1 : k * GPC + GPC + 1]
                 - cutg[k * GPC : k * GPC + GPC]).astype(np.float32)
        rec_k = (1.0 / np.maximum(cnt_k, 1.0)).astype(np.float32)
        con_k = np.empty((128, NCON), np.float32)
        con_k[:, 0] = rec_k
        con_k[:, 1 : 1 + D] = W_base[:, 0][None, :]
        con_k[:, 1 + D :] = conv[None, :]
        in_maps.append(dict(
            xcc=xcc_p, brs=brs, con_b=con_k,
        ))

    nc = _build_program(S, C, R)

    kw = {}
    if _profile is not None:
        kw = dict(trace=True, **_profile)
    res = run_bass_kernel_spmd(nc, in_maps, list(range(N_CORES)), **kw)

    out = np.concatenate([res.results[k]["out"] for k in range(N_CORES)], axis=0)
    if _profile is not None:
        return out.astype(np.float32), res
    return out.astype(np.float32)


# revision 29
# speedup vs baseline: 1.1173x; 1.1173x over previous
"""Trainium2 Bass kernel for nn_ClockAwareGNN (segment_reduce).

Model (reference, fp32):
    gp   = segment_mean(x, batch) @ W_base + b_base            # [B, 1]
    h    = relu(clock @ W1 + b1) @ W2 + b2                     # [N, 16]
    cp   = segment_mean(h, batch)                              # [B, 16]
    out  = relu([gp | cp] @ W3 + b3) @ W4 + b4                 # [B, 1]

Everything after the segment reductions is affine in per-graph quantities, so
the heavy per-node work collapses to fused segment sums:
    Sx[g]  = sum of x rows in graph g          (128 cols)
    Sr[g]  = sum of r rows in graph g          (R cols)
where r is either the raw clock (R=1; exact when b1 == 0 and clock >= 0 since
relu(c*W1) == c*relu(W1) elementwise for c >= 0) or the host-computed
relu(clock @ W1 + b1) (R=16 fallback).  Graph node counts (and their
reciprocals) come from the sorted batch vector on the host, so no count
column is shipped.

Device strategy (per core, 8-way data-parallel by graph):
  - the whole payload is ONE fp8e4m3 tensor [x | r_hi] -> 1 B/elem of
    x traffic (the 2e-2 rel-err gate leaves ~8x margin at fp8 precision).
  - nodes arrive as 128-row tiles; batch ids are sorted so each tile touches
    <= 2 graphs inside one 32-graph "window" (4 windows per core).
  - tiles are packed ROUND-ROBIN across the 4 windows, so consecutive
    matmuls target different 32-partition PE col-groups and run concurrently
    on different sub-arrays (col tiling), ~4x PE throughput.
  - the payload DRAM layout is partition-major ([128, n_tiles*C]) so a DMA
    block of any size is one plain 2D transfer with long contiguous
    per-partition lines (33 KB at 4 supers); ramped block sizes alternate
    between the two HWDGE rings (sync/SP + scalar/ACT).
  - DVE builds one-hot assign tiles [128 nodes, 32 graphs] per super-tile in
    one is_equal op (broadcast AP vs an iota pattern).
  - PE accumulates assign.T @ payload into PSUM [128 graphs, C] fp32.
  - short fused vector-engine epilogue reads PSUM directly and computes the
    folded per-graph MLP.
"""

import math
import sys
import types

import numpy as np
import ml_dtypes

import concourse.bass as bass
import concourse.bacc as bacc
import concourse.tile as tile
from concourse import mybir
from concourse.bass_utils import run_bass_kernel_spmd


def _ensure_axon_hooks():
    """bass_utils' trace path does `from antenv.axon_hooks import ...`;
    some agent images lack that submodule. Install it (with the real NTFF
    hook when available) so trace=True degrades gracefully instead of
    raising ModuleNotFoundError."""
    try:
        import antenv  # noqa: F401
        import antenv.axon_hooks  # noqa: F401
        return
    except ImportError:
        pass
    try:
        import antenv
    except ImportError:
        return
    mod = types.ModuleType("antenv.axon_hooks")
    state = {"hook": None}
    mod.set_axon_ntff_profile_hook = lambda h: state.__setitem__("hook", h)
    mod.get_axon_ntff_profile_hook = lambda: state["hook"]
    sys.modules["antenv.axon_hooks"] = mod
    antenv.axon_hooks = mod
    try:
        from trn_agent_boot.trn_boot import _ntff_profile_via_ctypes
        mod.set_axon_ntff_profile_hook(
            _ntff_profile_via_ctypes("/opt/axon/libaxon_pjrt.so"))
    except Exception:
        pass
    # the trace path also uploads the NEFF dir to a bucket; in zero-egress
    # containers that raises — fall back to the local path.
    try:
        import concourse.bass_utils as _bu
        _orig_upload = _bu.upload_artifacts

        def _safe_upload(tmpdir):
            try:
                return _orig_upload(tmpdir)
            except Exception:
                return str(tmpdir)

        _bu.upload_artifacts = _safe_upload
    except Exception:
        pass


_ensure_axon_hooks()

BF16 = ml_dtypes.bfloat16
F8 = ml_dtypes.float8_e4m3

N_CORES = 8
N_GRAPHS = 1024
D = 128                 # feature dim of x
GPC = N_GRAPHS // N_CORES   # graphs per core = 128
W = 32                  # one-hot window width (PSUM partition alignment unit)
WPC = GPC // W          # windows per core = 4
ST = 64                 # node-tiles per super-tile (assign-op granularity)
MAXBT = 2 * ST          # steady-state DMA block size in node-tiles


def _ring_blocks(n_tiles):
    """DMA blocks in node-tile units: ramp up (so the first matmuls aren't
    gated on a multi-MB transfer), big middle blocks (fewer HBM
    write-receipt jitter events at the consumption frontier), ramp down (so
    the last matmuls start right after the last bytes land). Rings alternate
    in consumption order; the scalar ring gets block 0 (the sync ring
    carries the batch-rel ids at kernel start), so head-of-line blocking is
    bounded by one block."""
    up = [32, 32, 64, 64, 128, 128]
    down = [128, 64, 32, 32]
    sizes = []
    left = n_tiles
    for v in up:
        if left - v >= sum(down):
            sizes.append(v)
            left -= v
    ndown = [v for v in down if v <= left]
    left -= sum(ndown)
    mid = []
    while left:
        bt = min(MAXBT, left)
        mid.append(bt)
        left -= bt
    sizes += mid + ndown
    out = []
    j0 = 0
    for i, bt in enumerate(sizes):
        out.append(((i + 1) % 2, j0, bt))
        j0 += bt
    return out


def _build_program(S, C, R):
    """Build the SPMD Bass/Tile program. Shapes are static; per-core data
    differences live entirely in the input tensors.

    S: number of super-tiles (each ST node-tiles of 128 nodes)
    C: fp8 payload column count = 128 + R
    """
    fp32 = mybir.dt.float32
    bf16 = mybir.dt.bfloat16
    f8 = mybir.dt.float8e4
    n_tiles = S * ST
    blocks = _ring_blocks(n_tiles)
    # combined epilogue-constants layout: [rec|wb|v1|m2|v0|w4|bb|b4]
    NCON = 1 + D + 32 + R * 32 + 32 + 32 + 1 + 1

    nc = bacc.Bacc("TRN2", target_bir_lowering=False, debug=False,
                   num_devices=N_CORES)

    BRH = min(4 * ST, S * ST)   # brall head: enough for the first 4 supers

    xcc = nc.dram_tensor("xcc", [128, n_tiles * C], f8, kind="ExternalInput").ap()
    brs = nc.dram_tensor("brs", [128, S * ST], bf16, kind="ExternalInput").ap()
    con_b = nc.dram_tensor("con_b", [128, NCON], fp32, kind="ExternalInput").ap()
    out_d = nc.dram_tensor("out", [128, 1], fp32, kind="ExternalOutput").ap()

    with tile.TileContext(nc) as tc:
        with (
            tc.tile_pool(name="consts", bufs=1) as cpool,
            tc.tile_pool(name="xin", bufs=4) as xpool,
            tc.tile_pool(name="assign", bufs=6) as apool,
            tc.tile_pool(name="epi", bufs=1) as epool,
            tc.tile_pool(name="ps", bufs=1, space="PSUM") as ppool,
        ):
            # ---- constants: iota is generated on the (otherwise idle)
            # GpSimd engine; the batch-rel ids stream on the sync ring with a
            # small head first so the first is_equal isn't gated on the full
            # transfer, while the scalar ring streams payload block 0.
            iota_t = cpool.tile([128, W], bf16, tag="iota")
            nc.gpsimd.iota(
                iota_t[:], pattern=[[1, W]], base=0, channel_multiplier=0,
                allow_small_or_imprecise_dtypes=True,
            )
            brall = cpool.tile([128, S * ST], bf16, tag="brall")
            nc.sync.dma_start(brall[:, 0:BRH], brs[:, 0:BRH])
            nc.sync.dma_start(brall[:, BRH:], brs[:, BRH:])
            # epilogue-only constants: issued on the scalar ring AFTER its
            # first payload block (they're needed only at the very end)
            con_t = cpool.tile([128, NCON], fp32, tag="con")
            rec_t = con_t[:, 0:1]
            wb_t = con_t[:, 1 : 1 + D]
            v1_t = con_t[:, 1 + D : 33 + D]
            m2_t = con_t[:, 33 + D : 33 + D + R * 32]
            v0_t = con_t[:, 33 + D + R * 32 : 65 + D + R * 32]
            w4_t = con_t[:, 65 + D + R * 32 : 97 + D + R * 32]
            bbt = con_t[:, 97 + D + R * 32 : 98 + D + R * 32]
            b4t = con_t[:, 98 + D + R * 32 : 99 + D + R * 32]

            psum = ppool.tile([128, C], fp32, tag="acc")

            # init matmul: zero weights x zero rhs, start=True claims the
            # whole bank's has_written bits so all later matmuls (start=False)
            # overwrite-on-first-touch / accumulate-after, independent of
            # window interleaving.
            zw = cpool.tile([128, 128], bf16, tag="zw")
            nc.vector.memset(zw[:], 0.0)
            zr = cpool.tile([128, C], bf16, tag="zr")
            nc.vector.memset(zr[:], 0.0)
            nc.tensor.matmul(psum[:, :], zw[:], zr[:], start=True, stop=False)

            # ---- main loop: 2D payload DMA blocks (tile-granular, ramped),
            # one-hot assign per super-tile, matmuls per node-tile.
            rings = (nc.sync, nc.scalar)
            asgs = [None] * S
            iota_bc = (iota_t[:].rearrange("p (o j) -> p o j", o=1)
                       .to_broadcast((128, ST, W)))
            con_sent = False
            for ring_idx, j0, bt in blocks:
                xt = xpool.tile([128, MAXBT * C], f8, tag="xt")
                L = bt * C
                rings[ring_idx].dma_start(
                    xt[:, 0:L], xcc[:, j0 * C : j0 * C + L])
                if not con_sent and ring_idx == 1:
                    nc.scalar.dma_start(con_t[:], con_b)
                    con_sent = True
                for i in range(j0, j0 + bt):
                    s = i // ST
                    if asgs[s] is None:
                        # asg[p, t, j] = (iota[j] == br[p, s*ST + t])
                        asg = apool.tile([128, ST * W], bf16, tag="asg")
                        nc.vector.tensor_tensor(
                            asg[:].rearrange("p (t j) -> p t j", j=W),
                            iota_bc,
                            brall[:, s * ST : (s + 1) * ST]
                                .rearrange("p (t o) -> p t o", o=1)
                                .to_broadcast((128, ST, W)),
                            op=mybir.AluOpType.is_equal,
                        )
                        asgs[s] = asg
                    w = i % WPC        # round-robin window -> PE col-group
                    last = i == n_tiles - 1
                    off = (i - j0) * C
                    nc.tensor.matmul(
                        psum[w * W : (w + 1) * W, 0:C],
                        asgs[s][:, (i % ST) * W : (i % ST + 1) * W],
                        xt[:, off : off + C],
                        start=False,
                        stop=last,
                        tile_position=(0, w * W),
                    )

            # ---- epilogue (per-graph folded MLP), vector-only, PSUM-direct
            # gp = (Sx @ W_base) * rec + b_base
            t1 = epool.tile([128, D], fp32, tag="t1")
            nc.vector.tensor_mul(t1[:], psum[:, 0:D], wb_t)
            gp = epool.tile([128, 1], fp32, tag="gp")
            nc.vector.tensor_reduce(gp[:], t1[:], axis=mybir.AxisListType.X,
                                    op=mybir.AluOpType.add)
            nc.vector.tensor_scalar(gp[:], gp[:], rec_t, bbt,
                                    op0=mybir.AluOpType.mult,
                                    op1=mybir.AluOpType.add)
            # mr = Sr * rec
            mr = epool.tile([128, R], fp32, tag="mr")
            nc.vector.tensor_scalar_mul(mr[:], psum[:, D : D + R], rec_t)

            # pre = gp*v1 + sum_j mr[:,j]*M2[j] + v0
            pre = epool.tile([128, 32], fp32, tag="pre")
            nc.vector.scalar_tensor_tensor(pre[:], v1_t, gp[:], v0_t,
                                           op0=mybir.AluOpType.mult,
                                           op1=mybir.AluOpType.add)
            for j in range(R):
                nc.vector.scalar_tensor_tensor(
                    pre[:], m2_t[:, j * 32 : (j + 1) * 32], mr[:, j : j + 1],
                    pre[:], op0=mybir.AluOpType.mult,
                    op1=mybir.AluOpType.add)
            # out = rowsum(relu(pre) * W4) + b4
            act = epool.tile([128, 32], fp32, tag="act")
            nc.vector.tensor_scalar_max(act[:], pre[:], 0.0)
            nc.vector.tensor_mul(act[:], act[:], w4_t)
            oo = epool.tile([128, 1], fp32, tag="oo")
            nc.vector.tensor_reduce(oo[:], act[:], axis=mybir.AxisListType.X,
                                    op=mybir.AluOpType.add)
            nc.vector.tensor_add(oo[:], oo[:], b4t)

            nc.scalar.dma_start(out_d, oo[:])

    nc.compile()
    return nc


def kernel(x, clock_period, batch, W_base, b_base, W1, b1, W2, b2, W3, b3, W4, b4,
           _profile=None):
    x = np.asarray(x, np.float32)
    clock = np.asarray(clock_period, np.float32).reshape(-1)
    batch = np.asarray(batch, np.int32)
    W_base = np.asarray(W_base, np.float32)
    W1 = np.asarray(W1, np.float32); b1 = np.asarray(b1, np.float32)
    W2 = np.asarray(W2, np.float32); b2 = np.asarray(b2, np.float32)
    W3 = np.asarray(W3, np.float32); b3 = np.asarray(b3, np.float32)
    W4 = np.asarray(W4, np.float32); b4 = np.asarray(b4, np.float32)
    hid = W1.shape[1]

    # r-path: exact algebraic fold when relu(c*W1 + b1) == c * relu(W1)
    fold = bool(np.all(b1 == 0.0)) and bool(clock.min() >= 0.0)
    if fold:
        R = 1
        r32 = clock[:, None]                                   # [N, 1]
        q = np.maximum(W1, 0.0) @ W2                           # [1, hid]
        M2 = q @ W3[1:, :]                                     # [1, 32]
        v0 = b2 @ W3[1:, :] + b3                               # [32]
    else:
        R = hid
        r32 = np.maximum(clock[:, None] @ W1 + b1, 0.0)        # [N, hid]
        M2 = W2 @ W3[1:, :]                                    # [hid, 32]
        v0 = b2 @ W3[1:, :] + b3

    C = D + R               # [x | r_hi], all fp8e4m3

    # ---- shard by graph; window padding so tile->window map is static ----
    cutw = np.searchsorted(batch, np.arange(0, N_GRAPHS + 1, W))
    cutg = np.searchsorted(batch, np.arange(0, N_GRAPHS + 1))
    win_nodes = np.diff(cutw)
    T_w = int(math.ceil(win_nodes.max() / 128.0))
    while (WPC * T_w) % ST:
        T_w += 1
    n_tiles = WPC * T_w
    S = n_tiles // ST
    Npad = n_tiles * 128

    xq = x.astype(F8)
    rhi = r32.astype(F8)

    in_maps = []
    NCON = 1 + D + 32 + R * 32 + 32 + 32 + 1 + 1
    conv = np.empty(NCON - 1 - D, np.float32)  # static part after [rec|wb]
    conv[0:32] = W3[0, :]
    conv[32 : 32 + R * 32] = M2.reshape(-1)
    conv[32 + R * 32 : 64 + R * 32] = v0
    conv[64 + R * 32 : 96 + R * 32] = W4[:, 0]
    conv[96 + R * 32] = float(b_base.reshape(-1)[0])
    conv[97 + R * 32] = float(b4.reshape(-1)[0])

    for k in range(N_CORES):
        xcc = np.zeros((Npad, C), F8)
        br = np.full(Npad, -1.0, BF16)
        for wi in range(WPC):
            gw = k * WPC + wi          # global window index
            s0, e0 = int(cutw[gw]), int(cutw[gw + 1])
            n = e0 - s0
            # window wi's rows live in tiles j = wi, wi+WPC, wi+2*WPC, ...
            # (round-robin across windows); build a row-index map for them.
            rows = (np.arange(n) // 128) * (WPC * 128) + wi * 128 + (np.arange(n) % 128)
            xcc[rows, 0:D] = xq[s0:e0]
            xcc[rows, D : D + R] = rhi[s0:e0]
            br[rows] = (batch[s0:e0] - gw * W).astype(BF16)
        brs = np.ascontiguousarray(br.reshape(S * ST, 128).T)
        # partition-major: each SBUF partition's whole run is contiguous
        xcc_p = np.ascontiguousarray(
            xcc.reshape(n_tiles, 128, C).transpose(1, 0, 2)
        ).reshape(128, n_tiles * C)
        cnt_k = (cutg[k * GPC + 1 : k * GPC + GPC + 1]
                 - cutg[k * GPC : k * GPC + GPC]).astype(np.float32)
        rec_k = (1.0 / np.maximum(cnt_k, 1.0)).astype(np.float32)
        con_k = np.empty((128, NCON), np.float32)
        con_k[:, 0] = rec_k
        con_k[:, 1 : 1 + D] = W_base[:, 0][None, :]
        con_k[:, 1 + D :] = conv[None, :]
        in_maps.append(dict(
            xcc=xcc_p, brs=brs, con_b=con_k,
        ))

    nc = _build_program(S, C, R)

    kw = {}
    if _profile is not None:
        kw = dict(trace=True, **_profile)
    res = run_bass_kernel_spmd(nc, in_maps, list(range(N_CORES)), **kw)

    out = np.concatenate([res.results[k]["out"] for k in range(N_CORES)], axis=0)
    if _profile is not None:
        return out.astype(np.float32), res
    return out.astype(np.float32)
